# revision 59
# baseline (speedup 1.0000x reference)
"""DOTA mix E-step (vq_codebook) on 8 TRN2 NeuronCores.

out[b,k,m] = gamma_class[b,k] * softmax_m(-0.5*(log_det+maha) + log_pi)

Implicit-reference formulation: softmax over modes is shift-invariant, so
each class pins one reference mode r (the one with the largest constant
term) at logit 0 and the GEMM computes only the c-1 DIFFERENCE logits

  l''[b,j] = x2 . (W1_m - W1_r + dlc) + x . (W2_m - W2_r)

with W1 = -0.5/var, W2 = mu/var and the per-column constant dlc folded
uniformly into W1diff (legal since sum_d x^2 = 1 for unit-norm x). Then

  s = 1 + sum_j exp(l''), coef = gamma / s,
  out_m = coef * e_m (m != r), out_r = coef.

No-overflow guarantee: l'' <= max_d(W1diff) + ||W2diff|| (x2 lies on the
simplex, |x| = 1), checked on host to stay well under exp's f32 range.
This cuts packed GEMM columns from sum(c) to sum(c-1) (~438/core), which
fits ONE PSUM bank: no second column tile, no LDWEIGHTS stalls.

Classes are bucketed by width c-1, rounded to multiples of 8 by promoting
classes from the next-lower pool (one wasted -20000 column each) so all
cores run one SPMD program. Count-1 classes are exact on host (resp = 1).
Post-GEMM work (reduce / +1 / recip / coef / scale) runs once per 4-chunk
group to amortize per-instruction overhead; outputs stream back in f16.
"""

import sys

import ml_dtypes
import numpy as np

sys.path.insert(0, "/opt/trn_rl_repo")

import concourse.bass as bass
import concourse.mybir as mybir
import concourse.tile as tile
from concourse import bacc, bass_utils

F32 = mybir.dt.float32
F16 = mybir.dt.float16
F8 = mybir.dt.float8e4

FP8_W1 = True      # x2@W1diff GEMM in fp8 DoubleRow (2 cols/cycle)
X2S = 8.0          # x2 scaled by X2S^2=64 into e4m3's normal range
NVP = None         # set in build: nv padded to mult of 16 for DoubleRow APs

B, K, M, D = 4096, 1000, 8, 512
NCORES = 8
NB = B // 128             # 32 batch chunks of 128 rows
GROUPS = (4, 4, 8, 8, 4, 2, 1, 1)  # small first groups start the DVE
                                   # post-pipeline early; tapered tail
EPS_REG = 1e-3
PAD_LOGIT = -20000.0      # exp -> 0 for promoted/dummy columns
MAX_LOGIT = 75.0          # overflow guard for exp in f32


def build_bass(buckets):
    """buckets: tuple of (width, n_classes_per_core) for widths 1..7."""
    nv = sum(w * n for w, n in buckets)       # packed diff columns per core
    kc = sum(n for _, n in buckets)           # packed classes per core
    assert nv <= 512, nv
    GMAX = max(GROUPS)
    NQ = NB // 4              # x loaded in quads of 4 chunks

    nvp = (nv + 15) // 16 * 16    # 16-elem-aligned i-stride for DoubleRow

    nc = bacc.Bacc("TRN2", debug=False, target_bir_lowering=False)
    # x stored (r, d, bc, j) so a 4-chunk load reads 1KB-contiguous segments;
    # x^2 is computed on device (saves 4MB of DMA)
    xt = nc.dram_tensor("xt", (4, 128, NB, 128), F16, kind="ExternalInput")
    if FP8_W1:
        w1 = nc.dram_tensor("w1", (2, 128, 2 * nvp), F8, kind="ExternalInput")
        x2t = nc.dram_tensor("x2t", (4, 128, NB, 128), F8,
                             kind="ExternalInput")
    else:
        w1 = nc.dram_tensor("w1", (4, 128, nv), F16, kind="ExternalInput")
        x2t = None
    w2 = nc.dram_tensor("w2", (4, 128, nv), F16, kind="ExternalInput")
    gam = nc.dram_tensor("gam", (128, NB * kc), F16, kind="ExternalInput")
    out = nc.dram_tensor("out", (B, nv), F16, kind="ExternalOutput")
    cf = nc.dram_tensor("cf", (B, kc), F16, kind="ExternalOutput")
    warm = nc.dram_tensor("warm", (128, 128), F32, kind="ExternalOutput")

    xt_ap, gam_ap, out_ap, cf_ap = (xt.ap(), gam.ap(), out.ap(), cf.ap())

    with tile.TileContext(nc) as tc:
        with (
            tc.tile_pool(name="wpool", bufs=1) as wpool,
            tc.tile_pool(name="xpool", bufs=3) as xpool,
            tc.tile_pool(name="ppool", bufs=4, space="PSUM") as ppool,
            tc.tile_pool(name="epool", bufs=3) as epool,
            tc.tile_pool(name="spool", bufs=3) as spool,
            tc.tile_pool(name="opool", bufs=3) as opool,
        ):
            # warmup weights first so the HAM warmup can start immediately
            wz = wpool.tile([128, 128], F16, tag="warmz")
            nc.gpsimd.memset(wz[:], 0.0)

            # weight tiles; DMAs spread over the two free queues (scalar's
            # ring carries the per-chunk x loads) so they land fast
            w1t, w2t = [], []
            if FP8_W1:
                for r in range(2):
                    t = wpool.tile([128, 2 * nvp], F8, tag=f"w1_{r}")
                    nc.sync.dma_start(t[:], w1.ap()[r])
                    w1t.append(t)
            else:
                for r in range(4):
                    t = wpool.tile([128, nv], F16, tag=f"w1_{r}")
                    nc.sync.dma_start(t[:], w1.ap()[r])
                    w1t.append(t)
            for r in range(4):
                t = wpool.tile([128, nv], F16, tag=f"w2_{r}")
                nc.gpsimd.dma_start(t[:], w2.ap()[r])
                w2t.append(t)

            # per-group gamma slices, loaded 2 groups ahead (a single bulk
            # gamma DMA gets hoisted ahead of the weight DMAs and stalls the
            # first chunks)
            NG = len(GROUPS)
            gstart = [sum(GROUPS[:i]) for i in range(NG)]
            gtiles = {}

            def load_gam(g):
                if g >= NG:
                    return
                g0, Gk = gstart[g], GROUPS[g]
                t = wpool.tile([128, GMAX * kc], F16, tag=f"gam{g}")
                nc.gpsimd.dma_start(t[:, 0:Gk * kc],
                                    gam_ap[:, g0 * kc:(g0 + Gk) * kc])
                gtiles[g] = t

            # HAM warmup: dummy matmuls while DMAs land, so the real GEMM
            # starts at 2.4 GHz instead of 1.2. One accumulation group so
            # consecutive MMs pipeline at N cycles instead of paying a full
            # fill+drain each.
            # ~100 MMs ≈ 7-8us: keeps the PE busy (and HAM warm) while the
            # weight/x DMAs are serviced, so real chunks start warm with
            # weights resident
            NWARM = 112
            wps = ppool.tile([128, 512], F32, tag="wps")
            for i in range(NWARM):
                nc.tensor.matmul(wps[:, 0:128], lhsT=wz[:], rhs=wz[:],
                                 start=(i == 0), stop=(i == NWARM - 1))
            wsb = wpool.tile([128, 128], F32, tag="warmsb")
            nc.vector.tensor_copy(wsb[:], wps[:, 0:128])
            nc.sync.dma_start(warm.ap()[:, :], wsb[:])

            # x quad loads (4 chunks each) on the scalar ring, one quad of
            # prefetch; x^2 computed on the scalar engine from the loaded x
            xtiles = {}

            def load_quad(p):
                if p >= NQ:
                    return
                xq = xpool.tile([128, 2048], F16, tag="xb")
                nc.scalar.dma_start(
                    xq[:].rearrange("p (r c j) -> p r c j", r=4, c=4),
                    xt_ap[:, :, 4 * p:4 * p + 4].rearrange(
                        "r p c j -> p r c j"))
                if FP8_W1:
                    # x^2 comes pre-squared in fp8 from the host
                    x2q = xpool.tile([128, 2048], F8, tag="x2b")
                    nc.scalar.dma_start(
                        x2q[:].rearrange("p (r c j) -> p r c j", r=4, c=4),
                        x2t.ap()[:, :, 4 * p:4 * p + 4].rearrange(
                            "r p c j -> p r c j"))
                else:
                    x2q = xpool.tile([128, 2048], F16, tag="x2b")
                    nc.scalar.square(x2q[:], xq[:])
                xtiles[p] = (xq, x2q)

            load_quad(0)
            load_gam(0)

            for g in range(NG):
                g0, Gk = gstart[g], GROUPS[g]
                load_gam(g + 1)
                ew = epool.tile([128, GMAX * nv], F32, tag="ew")
                ssum = spool.tile([128, GMAX * kc], F32, tag="ssum")

                def emit_reduces(c0, c1):
                    # reduce chunks [c0, c1) of this group into ssum
                    ewc = ew[:, c0 * nv:c1 * nv].rearrange(
                        "p (c v) -> p c v", v=nv)
                    ssc = ssum[:, c0 * kc:c1 * kc].rearrange(
                        "p (c k) -> p c k", c=c1 - c0)
                    off = koff = 0
                    for w, n in buckets:
                        nc.vector.reduce_sum(
                            ssc[:, :, koff:koff + n],
                            ewc[:, :, off:off + n * w].rearrange(
                                "p c (k m) -> p c k m", m=w),
                            axis=mybir.AxisListType.X)
                        off += n * w
                        koff += n

                half = Gk // 2 if Gk >= 4 else Gk
                for q in range(Gk):
                    bc = g0 + q
                    if bc % 4 == 0:
                        load_quad(bc // 4 + 1)
                        xq, x2q = xtiles.pop(bc // 4)
                    cq = (bc % 4) * 128
                    ps = ppool.tile([128, 512], F32, tag="ps")
                    if FP8_W1:
                        x2v = x2q[:].rearrange("p (r c j) -> p r c j",
                                               r=4, c=4)
                        for pr in range(2):
                            nc.tensor.matmul(
                                ps[:, 0:nv],
                                lhsT=x2v[:, 2 * pr:2 * pr + 2, bc % 4, :],
                                rhs=w1t[pr][:].rearrange(
                                    "p (i v) -> p i v", i=2)[:, :, 0:nv],
                                start=(pr == 0), stop=False,
                                perf_mode=mybir.MatmulPerfMode.DoubleRow)
                    else:
                        for r in range(4):
                            nc.tensor.matmul(
                                ps[:, 0:nv],
                                lhsT=x2q[:, r * 512 + cq:r * 512 + cq + 128],
                                rhs=w1t[r][:], start=(r == 0), stop=False)
                    for r in range(4):
                        nc.tensor.matmul(
                            ps[:, 0:nv],
                            lhsT=xq[:, r * 512 + cq:r * 512 + cq + 128],
                            rhs=w2t[r][:], start=False, stop=(r == 3))
                    nc.scalar.activation(ew[:, q * nv:(q + 1) * nv],
                                         ps[:, 0:nv],
                                         mybir.ActivationFunctionType.Exp)
                    if q == half - 1 and half < Gk:
                        # first-half reduces start while the second half's
                        # GEMM runs, cutting the end-of-group chain latency
                        emit_reduces(0, half)

                emit_reduces(half if half < Gk else 0, Gk)
                ew3 = ew[:, 0:Gk * nv].rearrange("p (c v) -> p c v", v=nv)
                nc.vector.tensor_scalar_add(
                    ssum[:, 0:Gk * kc], ssum[:, 0:Gk * kc], 1.0)
                rec = spool.tile([128, GMAX * kc], F32, tag="rec")
                nc.vector.reciprocal_approx_fast(
                    rec[:, 0:Gk * kc], ssum[:, 0:Gk * kc])
                coef = spool.tile([128, GMAX * kc], F32, tag="coef")
                mul_eng = nc.vector if g >= NG - 3 else nc.gpsimd
                mul_eng.tensor_mul(
                    coef[:, 0:Gk * kc], rec[:, 0:Gk * kc],
                    gtiles.pop(g)[:, 0:Gk * kc])
                cfo = spool.tile([128, GMAX * kc], F16, tag="cfo")
                nc.scalar.activation(cfo[:, 0:Gk * kc], coef[:, 0:Gk * kc],
                                     mybir.ActivationFunctionType.Copy)

                o4 = opool.tile([128, GMAX * nv], F16, tag="o4")
                o3 = o4[:, 0:Gk * nv].rearrange("p (c v) -> p c v", v=nv)
                c3 = coef[:, 0:Gk * kc].rearrange("p (c k) -> p c k", c=Gk)
                off = koff = 0
                for w, n in buckets:
                    e4 = ew3[:, :, off:off + n * w].rearrange(
                        "p c (k m) -> p c k m", m=w)
                    ob = o3[:, :, off:off + n * w].rearrange(
                        "p c (k m) -> p c k m", m=w)
                    cb = c3[:, :, koff:koff + n].rearrange(
                        "p c (k one) -> p c k one", one=1)
                    e4b, cbb = bass.broadcast_tensor_aps(e4, cb)
                    # GPSIMD is slow per-op on these strided broadcast APs
                    # (~1.5us) but runs in parallel during the body; near the
                    # tail everything goes to DVE so nothing spills past the
                    # last matmul.
                    # DVE's budget is consumed by the (DVE-only) reduces and
                    # reciprocal chain; all body scaling goes to GPSIMD
                    eng = nc.vector if g >= NG - 3 else nc.gpsimd
                    eng.tensor_tensor(ob, e4b, cbb, op=mybir.AluOpType.mult)
                    off += n * w
                    koff += n
                rows = slice(g0 * 128, (g0 + Gk) * 128)
                nc.sync.dma_start(
                    out_ap[rows, :].rearrange("(c p) j -> p c j", p=128),
                    o4[:, 0:Gk * nv].rearrange("p (c j) -> p c j", c=Gk))
                cf_eng = nc.sync if g >= NG - 3 else nc.gpsimd
                cf_eng.dma_start(
                    cf_ap[rows, :].rearrange("(c p) k -> p c k", p=128),
                    cfo[:, 0:Gk * kc].rearrange("p (c k) -> p c k", c=Gk))

    nc.compile()
    return nc


def _layout(mask):
    """Bucket classes by diff-width w = count-1 (count-1 classes are host
    handled); round each bucket to a multiple of NCORES by promoting classes
    from the next-lower pool (cost: 1 wasted column each); remaining gaps in
    the w=1 bucket get dummies (-1)."""
    counts = np.asarray(mask, bool).sum(-1).astype(int)     # (K,)
    pools = {w: list(np.where(counts == w + 1)[0]) for w in range(1, M)}
    entries = []
    for w in range(M - 1, 0, -1):
        ids = pools[w]
        pools[w] = []
        pad = (-len(ids)) % NCORES
        if pad and w > 1 and len(pools[w - 1]) >= pad:
            ids += pools[w - 1][:pad]
            pools[w - 1] = pools[w - 1][pad:]
        elif pad:
            ids += [-1] * pad
        if ids:
            entries.append((w, ids))
    entries.sort()
    per_core = [[] for _ in range(NCORES)]
    buckets = []
    for w, ids in entries:
        n = len(ids) // NCORES
        buckets.append((w, n))
        for c in range(NCORES):
            per_core[c].append((w, ids[c * n:(c + 1) * n]))
    ones = np.where(counts == 1)[0]
    return tuple(buckets), per_core, ones


def prep_inputs(x, gamma_class, mu_pad, var_pad, pi_pad, mask):
    x = np.asarray(x, np.float32)
    gamma_class = np.asarray(gamma_class, np.float32)
    mask = np.asarray(mask, bool)
    counts = mask.sum(-1).astype(int)

    var = np.clip(np.asarray(var_pad, np.float64) + EPS_REG, 1e-8, None)
    inv = 1.0 / var
    W1 = -0.5 * inv                                    # (K, M, D)
    W2 = np.asarray(mu_pad, np.float64) * inv
    logdet = np.log(var).sum(-1)
    muinvmu = (np.asarray(mu_pad, np.float64) * W2).sum(-1)
    logpi = np.where(mask, np.log(np.asarray(pi_pad, np.float64) + 1e-10),
                     -np.inf)
    lc = -0.5 * logdet - 0.5 * muinvmu + logpi          # (K, M)

    lc_valid = np.where(mask, lc, -np.inf)
    ref = np.argmax(lc_valid, axis=1)                   # (K,)

    def class_bound(k, r):
        c = counts[k]
        ms = [m for m in range(c) if m != r]
        if not ms:
            return -np.inf
        dW1 = W1[k, ms] - W1[k, r] + (lc[k, ms] - lc[k, r])[:, None]
        dW2 = W2[k, ms] - W2[k, r]
        return (dW1.max(-1) + np.sqrt((dW2 ** 2).sum(-1))).max()

    # overflow guard: exp stays finite in f32; re-pick ref if needed
    for k in np.where(counts >= 2)[0]:
        if class_bound(k, ref[k]) > MAX_LOGIT:
            cand = [(class_bound(k, r), r) for r in range(counts[k])]
            bd, r = min(cand)
            if bd > MAX_LOGIT:
                raise ValueError(f"class {k}: logit bound {bd:.1f} > "
                                 f"{MAX_LOGIT}; scheme unsafe")
            ref[k] = r

    buckets, per_core, ones = _layout(mask)
    nv = sum(w * n for w, n in buckets)
    kc = sum(n for _, n in buckets)

    # layout (r, d_in_block, bc, j): xt[r, p, bc, j] = x[bc*128+j, r*128+p]
    x16 = x.astype(np.float16)
    xtb = np.ascontiguousarray(
        x16.reshape(NB, 128, 4, 128).transpose(2, 3, 0, 1))
    x2tb = None
    if FP8_W1:
        x2tb = np.ascontiguousarray(
            np.clip((X2S * x16.astype(np.float64)) ** 2, 0, 240)
            .reshape(NB, 128, 4, 128).transpose(2, 3, 0, 1)
            .astype(ml_dtypes.float8_e4m3))

    in_maps, metas = [], []
    for cidx in range(NCORES):
        # unused/promoted cols: every element PAD_LOGIT/D so the folded
        # constant sums to PAD_LOGIT (Sx2=1) -> exp ~ 1e-17, never scattered
        w1c = np.full((nv, D), PAD_LOGIT / D, np.float64)
        w2c = np.zeros((nv, D), np.float64)
        gcols = np.zeros((B, kc), np.float32)
        col_cls = np.full(nv, -1, np.int64)
        col_mode = np.zeros(nv, np.int64)
        kcls = np.full(kc, -1, np.int64)
        kref = np.zeros(kc, np.int64)
        off = koff = 0
        for w, ids in per_core[cidx]:
            for k in ids:
                if k >= 0:
                    c, r = counts[k], ref[k]
                    ms = [m for m in range(c) if m != r]
                    nm = len(ms)
                    w1c[off:off + nm] = (W1[k, ms] - W1[k, r]
                                         + (lc[k, ms] - lc[k, r])[:, None])
                    w2c[off:off + nm] = W2[k, ms] - W2[k, r]
                    # promoted classes: unused cols stay at exp->0
                    col_cls[off:off + nm] = k
                    col_mode[off:off + nm] = ms
                    kcls[koff] = k
                    kref[koff] = r
                    gcols[:, koff] = gamma_class[:, k]
                off += w
                koff += 1
        # the PAD_LOGIT/D init makes unused cols sum to PAD_LOGIT via Sx2=1
        if FP8_W1:
            nvp = (nv + 15) // 16 * 16
            tmp = (w1c.T / (X2S * X2S)).reshape(2, 2, 128, nv) \
                .transpose(0, 2, 1, 3)                  # (pair, p, i, nv)
            w1pk = np.zeros((2, 128, 2, nvp), np.float64)
            w1pk[..., :nv] = tmp
            w1pk = np.clip(w1pk, -240, 240).reshape(2, 128, 2 * nvp) \
                .astype(ml_dtypes.float8_e4m3)
        else:
            w1pk = w1c.T.astype(np.float16).reshape(4, 128, nv)
        in_maps.append({
            "xt": xtb,
            **({"x2t": x2tb} if FP8_W1 else {}),
            "w1": np.ascontiguousarray(w1pk),
            "w2": np.ascontiguousarray(
                w2c.T.astype(np.float16).reshape(4, 128, nv)),
            "gam": np.ascontiguousarray(
                gcols.reshape(NB, 128, kc).transpose(1, 0, 2)
                .reshape(128, NB * kc).astype(np.float16)),
        })
        metas.append((col_cls, col_mode, kcls, kref))
    return in_maps, buckets, metas, ones


def scatter_core(out, packed, cfp, meta):
    """Scatter one core's packed (B, nv) diffs + (B, kc) coefs into out."""
    col_cls, col_mode, kcls, kref = meta
    real = col_cls >= 0
    out[:, col_cls[real], col_mode[real]] = packed[:, real]
    realk = kcls >= 0
    out[:, kcls[realk], kref[realk]] = cfp[:, realk]


_NC_CACHE = {}


def _get_nc(buckets):
    if buckets not in _NC_CACHE:
        _NC_CACHE[buckets] = build_bass(buckets)
    return _NC_CACHE[buckets]


def kernel(x, gamma_class, mu_pad, var_pad, pi_pad, mask, _trace=False):
    in_maps, buckets, metas, ones = prep_inputs(
        x, gamma_class, mu_pad, var_pad, pi_pad, mask)
    gamma_class = np.asarray(gamma_class, np.float32)
    out = np.zeros((B, K, M), np.float32)
    if len(ones):
        out[:, ones, 0] = gamma_class[:, ones]
    if not buckets:
        return out
    nc = _get_nc(buckets)
    res = bass_utils.run_bass_kernel_spmd(
        nc, in_maps, core_ids=list(range(NCORES)), trace=_trace)
    for cidx in range(NCORES):
        scatter_core(out, res.results[cidx]["out"].astype(np.float32),
                     res.results[cidx]["cf"].astype(np.float32), metas[cidx])
    if _trace:
        kernel.last_results = res
    return out
